# revision 2
# baseline (speedup 1.0000x reference)
# Trainium2 Bass kernel for LlamaAttention prefill with KV cache.
# Tensor-parallel over heads across 8 NeuronCores: core c owns KV head c and
# query heads 4c..4c+3. wo is applied row-parallel per core; the 8 partial
# [D, S] outputs are summed on the host (no device collectives).
#
# Self-contained: hardcodes shapes from the problem spec.
import numpy as np
import ml_dtypes

BF16 = ml_dtypes.bfloat16
S, D, HD, CACHE = 1024, 4096, 128, 1024
T = CACHE + S            # 2048 total keys
NH, NKV, HQ = 32, 8, 4   # 4 query heads per core, 1 kv head per core
DT = D // 128            # 32 contraction tiles
TT = T // 128            # 16 key tiles
SCALE = 1.0 / float(np.sqrt(HD))

_BUILT = {}


def _build():
    """Trace + finalize the Bass program (same program for all 8 cores)."""
    import contextlib
    import concourse.bass as bass
    import concourse.tile as tile
    from concourse import bacc, mybir
    from concourse.masks import make_identity

    f32, bf, f32r = mybir.dt.float32, mybir.dt.bfloat16, mybir.dt.float32r
    EXP = mybir.ActivationFunctionType.Exp

    nc = bacc.Bacc("TRN2", target_bir_lowering=False, debug=False, num_devices=8)

    # ---- DRAM I/O (per-core shards, host-pretransposed) ----
    hT = nc.dram_tensor("hT", (D, S), bf, kind="ExternalInput")          # hidden^T
    wqT = nc.dram_tensor("wqT", (D, HQ * HD), bf, kind="ExternalInput")  # per-head cols
    wkT = nc.dram_tensor("wkT", (D, HD), bf, kind="ExternalInput")
    wvT = nc.dram_tensor("wvT", (D, HD), bf, kind="ExternalInput")
    kcache = nc.dram_tensor("kcache", (HD, CACHE), f32, kind="ExternalInput")
    vcache = nc.dram_tensor("vcache", (CACHE, HD), f32, kind="ExternalInput")
    woT = nc.dram_tensor("woT", (HQ * HD, D), bf, kind="ExternalInput")  # wo slice^T
    cosF = nc.dram_tensor("cosF", (HD, S), f32, kind="ExternalInput")    # [cos; cos]
    sinF = nc.dram_tensor("sinF", (HD, S), f32, kind="ExternalInput")    # [-sin; sin]
    khn = nc.dram_tensor("khn", (HD, S), f32, kind="ExternalOutput")     # new k^T
    vhn = nc.dram_tensor("vhn", (S, HD), f32, kind="ExternalOutput")     # new v
    yprt = nc.dram_tensor("yprt", (D, S), f32, kind="ExternalOutput")    # partial y^T

    with tile.TileContext(nc) as tc:
        ctx = contextlib.ExitStack()
        # SBUF pools (per-partition KB in comments)
        big = ctx.enter_context(tc.tile_pool(name="big", bufs=8))     # 8x8K = 64K
        kpool = ctx.enter_context(tc.tile_pool(name="kpool", bufs=1))  # 8K
        vbfp = ctx.enter_context(tc.tile_pool(name="vbfp", bufs=1))   # 4K
        voutp = ctx.enter_context(tc.tile_pool(name="voutp", bufs=1))  # 4K
        f4k = ctx.enter_context(tc.tile_pool(name="f4k", bufs=8))     # 32K
        expp = ctx.enter_context(tc.tile_pool(name="expp", bufs=17))  # 34K
        ybfp = ctx.enter_context(tc.tile_pool(name="ybfp", bufs=4))   # 8K
        wst = ctx.enter_context(tc.tile_pool(name="wst", bufs=3))     # 6K
        smallp = ctx.enter_context(tc.tile_pool(name="smallp", bufs=1))
        trig = ctx.enter_context(tc.tile_pool(name="trig", bufs=2))   # 8K
        # PSUM pools: 4 + 2 + 2 = 8 banks
        qacc = ctx.enter_context(tc.tile_pool(name="qacc", bufs=1, space="PSUM"))
        stp = ctx.enter_context(tc.tile_pool(name="stp", bufs=4, space="PSUM"))
        ytp = ctx.enter_context(tc.tile_pool(name="ytp", bufs=2, space="PSUM"))

        with ctx:
            # ---- persistent small tiles ----
            ident = smallp.tile([128, 128], f32, tag="ident")
            make_identity(nc, ident)
            ones = smallp.tile([128, 1], bf, tag="ones")
            nc.vector.memset(ones[:], 1.0)
            cos_sb = trig.tile([128, S], f32, tag="trig")
            nc.sync.dma_start(out=cos_sb[:], in_=cosF.ap())
            sin_sb = trig.tile([128, S], f32, tag="trig")
            nc.sync.dma_start(out=sin_sb[:], in_=sinF.ap())

            # ---- hidden^T resident in SBUF: 8 chunks of [128, 4, 1024] bf16 ----
            hT_ap = hT.ap().rearrange("(o c p) s -> o p c s", o=8, c=4, p=128)
            h_tiles = []
            for o in range(8):
                t = big.tile([128, 4, S], bf, tag="big")
                nc.sync.dma_start(out=t[:], in_=hT_ap[o])
                h_tiles.append(t)

            # kT (f32r for matmul; fp32 bits, rounded) [128, T]
            kT_sb = kpool.tile([128, T], f32r, tag="kT")
            nc.sync.dma_start(out=kT_sb[:, :CACHE], in_=kcache.ap().bitcast(f32r))
            # v in [t, hd] layout, bf16, 16 tiles packed [128, 16, 128]
            v_bf = vbfp.tile([128, TT, 128], bf, tag="vbf")
            nc.gpsimd.dma_start(
                out=v_bf[:, : CACHE // 128, :],
                in_=vcache.ap().rearrange("(c p) e -> p c e", p=128),
            )

            def rope(dst, src_ps, tmp_pool):
                """dst[:64] = s[:64]c - s[64:]sn ; dst[64:] = s[:64]sn + s[64:]c
                using host-prepared cosF=[c;c], sinF=[-sn;sn]:
                  dst = src*cosF_sw? -- direct half ops (cross-base DVE is fine)."""
                t1 = tmp_pool.tile([128, S], f32, tag="f4k")
                # t1[:64] = src[64:] * sin ; t1[64:] = src[:64] * sin(second half rows)
                nc.vector.tensor_mul(out=t1[0:64, :], in0=src_ps[64:128, :], in1=sin_sb[0:64, :])
                nc.vector.tensor_mul(out=t1[64:128, :], in0=src_ps[0:64, :], in1=sin_sb[64:128, :])
                # dst = src * cos + t1   (sin rows 0:64 hold -sin)
                t2 = tmp_pool.tile([128, S], f32, tag="f4k")
                nc.vector.tensor_mul(out=t2[:], in0=src_ps[:], in1=cos_sb[:])
                nc.vector.tensor_add(out=dst, in0=t2[:], in1=t1[:])

            # ---- K projection ----
            def stream_w(dram_ap, g):
                wt = wst.tile([128, 8, 128], bf, tag="wst")
                nc.sync.dma_start(
                    out=wt[:],
                    in_=dram_ap.rearrange("(g c p) e -> g p c e", g=4, c=8, p=128)[g],
                )
                return wt

            def proj_pass(w_dram_ap, acc):
                for g in range(4):
                    wt = stream_w(w_dram_ap, g)
                    for c8 in range(8):
                        dt = g * 8 + c8
                        o, cc = dt // 4, dt % 4
                        st, sp = dt == 0, dt == DT - 1
                        nc.tensor.matmul(acc[:, 0:512], lhsT=wt[:, c8, :],
                                         rhs=h_tiles[o][:, cc, 0:512], start=st, stop=sp)
                        nc.tensor.matmul(acc[:, 512:1024], lhsT=wt[:, c8, :],
                                         rhs=h_tiles[o][:, cc, 512:1024], start=st, stop=sp)

            k_ps = qacc.tile([128, S], f32, tag="qacc")
            proj_pass(wkT.ap(), k_ps)
            rope(kT_sb[:, CACHE:], k_ps, f4k)
            nc.sync.dma_start(out=khn.ap(), in_=kT_sb[:, CACHE:].bitcast(f32))

            # ---- V projection + transpose ----
            v_ps = qacc.tile([128, S], f32, tag="qacc")
            proj_pass(wvT.ap(), v_ps)
            vT_sb = f4k.tile([128, S], f32, tag="f4k")
            nc.vector.tensor_copy(out=vT_sb[:], in_=v_ps[:])
            vout_sb = voutp.tile([128, S // 128, 128], f32, tag="vout")
            for i in range(S // 128):
                vp = stp.tile([128, 512], f32, tag="stp")
                nc.tensor.transpose(vp[:, 0:128], vT_sb[:, i * 128:(i + 1) * 128], ident)
                nc.vector.tensor_copy(out=v_bf[:, CACHE // 128 + i, :], in_=vp[:, 0:128])
                nc.vector.tensor_copy(out=vout_sb[:, i, :], in_=vp[:, 0:128])
            nc.sync.dma_start(
                out=vhn.ap().rearrange("(c p) e -> p c e", p=128), in_=vout_sb[:]
            )

            # ---- Q projections (per head) + attention ----
            q_tiles = {}

            def q_pass(h):
                acc = qacc.tile([128, S], f32, tag="qacc")
                proj_pass(wqT.ap()[:, h * HD:(h + 1) * HD], acc)
                qT = f4k.tile([128, S], f32r, tag="f4k")
                rope(qT[:], acc, f4k)
                q_tiles[h] = qT

            y_tiles = {}

            def attention(h):
                qT = q_tiles[h]
                ets = []
                for tt in range(TT):
                    st_a = stp.tile([128, 512], f32, tag="stp")
                    st_b = stp.tile([128, 512], f32, tag="stp")
                    lt = kT_sb[:, tt * 128:(tt + 1) * 128]
                    nc.tensor.matmul(st_a[:], lhsT=lt, rhs=qT[:, 0:512], start=True, stop=True)
                    nc.tensor.matmul(st_b[:], lhsT=lt, rhs=qT[:, 512:1024], start=True, stop=True)
                    eT = expp.tile([128, S], bf, tag="expp")
                    nc.scalar.activation(out=eT[:, 0:512], in_=st_a[:], func=EXP, scale=SCALE)
                    nc.scalar.activation(out=eT[:, 512:1024], in_=st_b[:], func=EXP, scale=SCALE)
                    ets.append(eT)
                # AV accumulation
                yt_a = ytp.tile([128, 512], f32, tag="ytp")
                yt_b = ytp.tile([128, 512], f32, tag="ytp")
                for tt in range(TT):
                    st, sp = tt == 0, tt == TT - 1
                    nc.tensor.matmul(yt_a[:], lhsT=v_bf[:, tt, :], rhs=ets[tt][:, 0:512], start=st, stop=sp)
                    nc.tensor.matmul(yt_b[:], lhsT=v_bf[:, tt, :], rhs=ets[tt][:, 512:1024], start=st, stop=sp)
                # softmax sums via ones-matmul (half a then b through one slot each)
                recip = f4k.tile([1, S], f32, tag="f4k")
                for half, (lo, hi) in enumerate([(0, 512), (512, 1024)]):
                    sm = stp.tile([128, 512], f32, tag="stp")
                    for tt in range(TT):
                        nc.tensor.matmul(sm[0:1, :], lhsT=ones[:], rhs=ets[tt][:, lo:hi],
                                         start=tt == 0, stop=tt == TT - 1)
                    nc.vector.reciprocal(out=recip[:, lo:hi], in_=sm[0:1, :])
                bc = f4k.tile([128, S], f32, tag="f4k")
                nc.gpsimd.partition_broadcast(bc[:], recip[:])
                ybf = ybfp.tile([128, S], bf, tag="ybf")
                nc.vector.tensor_mul(out=ybf[:, 0:512], in0=yt_a[:], in1=bc[:, 0:512])
                nc.vector.tensor_mul(out=ybf[:, 512:1024], in0=yt_b[:], in1=bc[:, 512:1024])
                y_tiles[h] = ybf

            # interleave Q-projections ahead of attention to keep PE dense
            q_pass(0)
            q_pass(1)
            attention(0)
            q_pass(2)
            attention(1)
            q_pass(3)
            attention(2)
            attention(3)

            # ---- output projection (row-parallel partial) ----
            wo_tiles = []
            woT_ap = woT.ap().rearrange("(f p) d -> f p d", f=4, p=128)
            for f in range(4):
                t = big.tile([128, 4 * S], bf, tag="big")  # same 8KB slot size
                nc.sync.dma_start(out=t[:], in_=woT_ap[f])
                wo_tiles.append(t)
            for dto in range(DT):
                pool = ytp if dto % 2 == 0 else stp
                yo_a = pool.tile([128, 512], f32, tag="ytp" if dto % 2 == 0 else "stp")
                yo_b = pool.tile([128, 512], f32, tag="ytp" if dto % 2 == 0 else "stp")
                for f in range(4):
                    lt = wo_tiles[f][:, dto * 128:(dto + 1) * 128]
                    st, sp = f == 0, f == 3
                    nc.tensor.matmul(yo_a[:], lhsT=lt, rhs=y_tiles[f][:, 0:512], start=st, stop=sp)
                    nc.tensor.matmul(yo_b[:], lhsT=lt, rhs=y_tiles[f][:, 512:1024], start=st, stop=sp)
                yo_sb = f4k.tile([128, S], f32, tag="f4k")
                nc.vector.tensor_copy(out=yo_sb[:, 0:512], in_=yo_a[:])
                nc.vector.tensor_copy(out=yo_sb[:, 512:1024], in_=yo_b[:])
                nc.sync.dma_start(out=yprt.ap()[dto * 128:(dto + 1) * 128, :], in_=yo_sb[:])

    nc.finalize()
    return nc


def _get_nc():
    if "nc" not in _BUILT:
        _BUILT["nc"] = _build()
    return _BUILT["nc"]


def _numpy_reference(hidden_states, freqs_cos, freqs_sin, atten_mask,
                     k_cache, v_cache, wq, wk, wv, wo):
    """Exact fp32 fallback (only used if atten_mask is nonzero)."""
    b, s, d = hidden_states.shape
    nh, hd, _ = wq.shape
    nkv = wk.shape[0]
    g = nh // nkv
    scale = float(hd) ** 0.5
    h = hidden_states.astype(np.float32)
    q = np.einsum("bsd,hed->bshe", h, wq)
    k = np.einsum("bsd,hed->bshe", h, wk)
    v = np.einsum("bsd,hed->bshe", h, wv)

    def rope(x):
        half = x.shape[-1] // 2
        xr, xi = x[..., :half], x[..., half:]
        c = freqs_cos[None, :, None, :]
        sn = freqs_sin[None, :, None, :]
        return np.concatenate([xr * c - xi * sn, xr * sn + xi * c], axis=-1)

    q, k = rope(q), rope(k)
    kh = np.concatenate([k_cache, k.transpose(0, 2, 3, 1)], axis=-1)
    vh = np.concatenate([v_cache, v.transpose(0, 2, 1, 3)], axis=2)
    qg = q.reshape(b, s, nkv, g, hd).transpose(0, 2, 3, 1, 4)
    scores = np.einsum("bngse,bnet->bngst", qg, kh) / scale + atten_mask
    m = scores.max(axis=-1, keepdims=True)
    p = np.exp(scores - m)
    p = p / p.sum(axis=-1, keepdims=True)
    yg = np.einsum("bngst,bnte->bngse", p, vh)
    y = yg.transpose(0, 3, 1, 2, 4).reshape(b, s, nh * hd)
    y = np.einsum("bsf,df->bsd", y, wo)
    return (y.astype(np.float32), kh.astype(np.float32), vh.astype(np.float32))


def kernel(hidden_states, freqs_cos, freqs_sin, atten_mask, k_cache, v_cache,
           wq, wk, wv, wo):
    hidden_states = np.asarray(hidden_states, np.float32)
    freqs_cos = np.asarray(freqs_cos, np.float32)
    freqs_sin = np.asarray(freqs_sin, np.float32)
    atten_mask = np.asarray(atten_mask, np.float32)
    k_cache = np.asarray(k_cache, np.float32)
    v_cache = np.asarray(v_cache, np.float32)
    wq = np.asarray(wq, np.float32)
    wk = np.asarray(wk, np.float32)
    wv = np.asarray(wv, np.float32)
    wo = np.asarray(wo, np.float32)

    if np.any(atten_mask):
        return _numpy_reference(hidden_states, freqs_cos, freqs_sin, atten_mask,
                                k_cache, v_cache, wq, wk, wv, wo)

    from concourse.bass_utils import run_bass_kernel_spmd

    nc = _get_nc()

    hT = np.ascontiguousarray(hidden_states[0].T).astype(BF16)        # [D, S]
    cosF = np.concatenate([freqs_cos.T, freqs_cos.T], axis=0).astype(np.float32)
    sinF = np.concatenate([-freqs_sin.T, freqs_sin.T], axis=0).astype(np.float32)

    in_maps = []
    for c in range(8):
        wqT = np.ascontiguousarray(
            wq[HQ * c:HQ * (c + 1)].transpose(2, 0, 1).reshape(D, HQ * HD)
        ).astype(BF16)
        wkT = np.ascontiguousarray(wk[c].T).astype(BF16)
        wvT = np.ascontiguousarray(wv[c].T).astype(BF16)
        woT = np.ascontiguousarray(wo[:, 512 * c:512 * (c + 1)].T).astype(BF16)
        in_maps.append({
            "hT": hT, "wqT": wqT, "wkT": wkT, "wvT": wvT,
            "kcache": np.ascontiguousarray(k_cache[0, c]),
            "vcache": np.ascontiguousarray(v_cache[0, c]),
            "woT": woT, "cosF": cosF, "sinF": sinF,
        })

    res = run_bass_kernel_spmd(nc, in_maps, core_ids=list(range(8)))

    y_acc = np.zeros((D, S), np.float32)
    kh = np.empty((1, NKV, HD, T), np.float32)
    vh = np.empty((1, NKV, T, HD), np.float32)
    kh[0, :, :, :CACHE] = k_cache[0]
    vh[0, :, :CACHE, :] = v_cache[0]
    for c in range(8):
        r = res.results[c]
        y_acc += r["yprt"]
        kh[0, c, :, CACHE:] = r["khn"]
        vh[0, c, CACHE:, :] = r["vhn"]
    y = np.ascontiguousarray(y_acc.T)[None]                           # [1, S, D]
    return (y, kh, vh)


# revision 4
# speedup vs baseline: 1.0360x; 1.0360x over previous
# Trainium2 Bass kernel for LlamaAttention prefill with KV cache.
# Tensor-parallel over heads across 8 NeuronCores: core c owns KV head c and
# query heads 4c..4c+3. wo is applied row-parallel per core; the 8 partial
# [D, S] outputs are summed on the host (no device collectives).
#
# Self-contained: hardcodes shapes from the problem spec.
import numpy as np
import ml_dtypes

BF16 = ml_dtypes.bfloat16
S, D, HD, CACHE = 1024, 4096, 128, 1024
T = CACHE + S            # 2048 total keys
NH, NKV, HQ = 32, 8, 4   # 4 query heads per core, 1 kv head per core
DT = D // 128            # 32 contraction tiles
TT = T // 128            # 16 key tiles
SCALE = 1.0 / float(np.sqrt(HD))

_BUILT = {}


def _build():
    """Trace + finalize the Bass program (same program for all 8 cores)."""
    import contextlib
    import concourse.bass as bass
    import concourse.tile as tile
    from concourse import bacc, mybir
    from concourse.masks import make_identity

    f32, bf, f32r = mybir.dt.float32, mybir.dt.bfloat16, mybir.dt.float32r
    EXP = mybir.ActivationFunctionType.Exp

    nc = bacc.Bacc("TRN2", target_bir_lowering=False, debug=False, num_devices=8)

    # ---- DRAM I/O (per-core shards, host-pretransposed) ----
    hT = nc.dram_tensor("hT", (D, S), bf, kind="ExternalInput")          # hidden^T
    wqT = nc.dram_tensor("wqT", (D, HQ * HD), bf, kind="ExternalInput")  # per-head cols
    wkT = nc.dram_tensor("wkT", (D, HD), bf, kind="ExternalInput")
    wvT = nc.dram_tensor("wvT", (D, HD), bf, kind="ExternalInput")
    kcache = nc.dram_tensor("kcache", (HD, CACHE), f32, kind="ExternalInput")
    vcache = nc.dram_tensor("vcache", (CACHE, HD), f32, kind="ExternalInput")
    woT = nc.dram_tensor("woT", (HQ * HD, D), bf, kind="ExternalInput")  # wo slice^T
    cosF = nc.dram_tensor("cosF", (HD, S), f32, kind="ExternalInput")    # [cos; cos]
    sinF = nc.dram_tensor("sinF", (HD, S), f32, kind="ExternalInput")    # [-sin; sin]
    khn = nc.dram_tensor("khn", (HD, S), f32, kind="ExternalOutput")     # new k^T
    vhn = nc.dram_tensor("vhn", (S, HD), f32, kind="ExternalOutput")     # new v
    yprt = nc.dram_tensor("yprt", (D, S), f32, kind="ExternalOutput")    # partial y^T

    with tile.TileContext(nc) as tc:
        ctx = contextlib.ExitStack()
        # SBUF pools (per-partition KB in comments)
        big = ctx.enter_context(tc.tile_pool(name="big", bufs=8))     # 8x8K = 64K
        kpool = ctx.enter_context(tc.tile_pool(name="kpool", bufs=1))  # 8K
        vbfp = ctx.enter_context(tc.tile_pool(name="vbfp", bufs=1))   # 4K
        voutp = ctx.enter_context(tc.tile_pool(name="voutp", bufs=1))  # 4K
        f4k = ctx.enter_context(tc.tile_pool(name="f4k", bufs=8))     # 32K
        expp = ctx.enter_context(tc.tile_pool(name="expp", bufs=17))  # 34K
        ybfp = ctx.enter_context(tc.tile_pool(name="ybfp", bufs=4))   # 8K
        wst = ctx.enter_context(tc.tile_pool(name="wst", bufs=6))     # 12K
        smallp = ctx.enter_context(tc.tile_pool(name="smallp", bufs=1))
        trig = ctx.enter_context(tc.tile_pool(name="trig", bufs=2))   # 8K
        # PSUM pools: 4 + 2 + 2 = 8 banks
        qacc = ctx.enter_context(tc.tile_pool(name="qacc", bufs=1, space="PSUM"))
        stp = ctx.enter_context(tc.tile_pool(name="stp", bufs=4, space="PSUM"))
        ytp = ctx.enter_context(tc.tile_pool(name="ytp", bufs=2, space="PSUM"))

        with ctx:
            # ---- persistent small tiles ----
            ident = smallp.tile([128, 128], f32, tag="ident")
            make_identity(nc, ident)
            ones = smallp.tile([128, 1], bf, tag="ones")
            nc.vector.memset(ones[:], 1.0)
            # hT chunk tiles (DMAs interleaved into the K-pass weight stream below
            # so the first matmul only waits on ~1.3MB, not the whole input set)
            hT_ap = hT.ap().rearrange("(o c p) s -> o p c s", o=8, c=4, p=128)
            h_tiles = [big.tile([128, 4, S], bf, tag="big", name=f"hT{_o}") for _o in range(8)]
            kT_sb = kpool.tile([128, T], f32r, tag="kT")
            v_bf = vbfp.tile([128, TT, 128], bf, tag="vbf")
            cos_sb = trig.tile([128, S], f32, tag="trig")
            sin_sb = trig.tile([128, S], f32, tag="trig")

            def rope(dst, src_ps, tmp_pool):
                """dst[:64] = s[:64]c - s[64:]sn ; dst[64:] = s[:64]sn + s[64:]c
                using host-prepared cosF=[c;c], sinF=[-sn;sn]:
                  dst = src*cosF_sw? -- direct half ops (cross-base DVE is fine)."""
                t1 = tmp_pool.tile([128, S], f32, tag="f4k")
                # t1[:64] = src[64:] * sin ; t1[64:] = src[:64] * sin(second half rows)
                nc.vector.tensor_mul(out=t1[0:64, :], in0=src_ps[64:128, :], in1=sin_sb[0:64, :])
                nc.vector.tensor_mul(out=t1[64:128, :], in0=src_ps[0:64, :], in1=sin_sb[64:128, :])
                # dst = src * cos + t1   (sin rows 0:64 hold -sin)
                t2 = tmp_pool.tile([128, S], f32, tag="f4k")
                nc.vector.tensor_mul(out=t2[:], in0=src_ps[:], in1=cos_sb[:])
                nc.vector.tensor_add(out=dst, in0=t2[:], in1=t1[:])

            # ---- K projection ----
            def stream_w(dram_ap, g):
                wt = wst.tile([128, 8, 128], bf, tag="wst")
                nc.sync.dma_start(
                    out=wt[:],
                    in_=dram_ap.rearrange("(g c p) e -> g p c e", g=4, c=8, p=128)[g],
                )
                return wt

            def proj_pass(w_dram_ap, acc):
                for g in range(4):
                    wt = stream_w(w_dram_ap, g)
                    for c8 in range(8):
                        dt = g * 8 + c8
                        o, cc = dt // 4, dt % 4
                        st, sp = dt == 0, dt == DT - 1
                        nc.tensor.matmul(acc[:, 0:512], lhsT=wt[:, c8, :],
                                         rhs=h_tiles[o][:, cc, 0:512], start=st, stop=sp)
                        nc.tensor.matmul(acc[:, 512:1024], lhsT=wt[:, c8, :],
                                         rhs=h_tiles[o][:, cc, 512:1024], start=st, stop=sp)

            k_ps = qacc.tile([128, S], f32, tag="qacc")
            for g in range(4):
                wt = stream_w(wkT.ap(), g)
                nc.sync.dma_start(out=h_tiles[2 * g][:], in_=hT_ap[2 * g])
                nc.sync.dma_start(out=h_tiles[2 * g + 1][:], in_=hT_ap[2 * g + 1])
                if g == 1:
                    # aux loads queued once the critical-path stream is rolling
                    nc.sync.dma_start(out=cos_sb[:], in_=cosF.ap())
                    nc.sync.dma_start(out=sin_sb[:], in_=sinF.ap())
                    nc.sync.dma_start(out=kT_sb[:, :CACHE], in_=kcache.ap().bitcast(f32r))
                    nc.gpsimd.dma_start(
                        out=v_bf[:, : CACHE // 128, :],
                        in_=vcache.ap().rearrange("(c p) e -> p c e", p=128),
                    )
                for c8 in range(8):
                    dt = g * 8 + c8
                    o, cc = dt // 4, dt % 4
                    st, sp = dt == 0, dt == DT - 1
                    nc.tensor.matmul(k_ps[:, 0:512], lhsT=wt[:, c8, :],
                                     rhs=h_tiles[o][:, cc, 0:512], start=st, stop=sp)
                    nc.tensor.matmul(k_ps[:, 512:1024], lhsT=wt[:, c8, :],
                                     rhs=h_tiles[o][:, cc, 512:1024], start=st, stop=sp)
            rope(kT_sb[:, CACHE:], k_ps, f4k)
            nc.sync.dma_start(out=khn.ap(), in_=kT_sb[:, CACHE:].bitcast(f32))

            # ---- V projection + transpose ----
            v_ps = qacc.tile([128, S], f32, tag="qacc")
            proj_pass(wvT.ap(), v_ps)
            vT_sb = f4k.tile([128, S], f32, tag="f4k")
            nc.vector.tensor_copy(out=vT_sb[:], in_=v_ps[:])
            vout_sb = voutp.tile([128, S // 128, 128], f32, tag="vout")
            for i in range(S // 128):
                vp = stp.tile([128, 512], f32, tag="stp")
                nc.tensor.transpose(vp[:, 0:128], vT_sb[:, i * 128:(i + 1) * 128], ident)
                nc.vector.tensor_copy(out=v_bf[:, CACHE // 128 + i, :], in_=vp[:, 0:128])
                nc.vector.tensor_copy(out=vout_sb[:, i, :], in_=vp[:, 0:128])
            nc.sync.dma_start(
                out=vhn.ap().rearrange("(c p) e -> p c e", p=128), in_=vout_sb[:]
            )

            # ---- Q projections (per head) + attention ----
            q_tiles = {}

            def q_pass(h):
                acc = qacc.tile([128, S], f32, tag="qacc")
                proj_pass(wqT.ap()[:, h * HD:(h + 1) * HD], acc)
                qT = f4k.tile([128, S], f32r, tag="f4k")
                rope(qT[:], acc, f4k)
                q_tiles[h] = qT

            y_tiles = {}

            def attention(h):
                qT = q_tiles[h]
                ets = []
                for tt in range(TT):
                    st_a = stp.tile([128, 512], f32, tag="stp")
                    st_b = stp.tile([128, 512], f32, tag="stp")
                    lt = kT_sb[:, tt * 128:(tt + 1) * 128]
                    nc.tensor.matmul(st_a[:], lhsT=lt, rhs=qT[:, 0:512], start=True, stop=True)
                    nc.tensor.matmul(st_b[:], lhsT=lt, rhs=qT[:, 512:1024], start=True, stop=True)
                    eT = expp.tile([128, S], bf, tag="expp")
                    nc.scalar.activation(out=eT[:, 0:512], in_=st_a[:], func=EXP, scale=SCALE)
                    nc.scalar.activation(out=eT[:, 512:1024], in_=st_b[:], func=EXP, scale=SCALE)
                    ets.append(eT)
                # AV accumulation
                yt_a = ytp.tile([128, 512], f32, tag="ytp")
                yt_b = ytp.tile([128, 512], f32, tag="ytp")
                for tt in range(TT):
                    st, sp = tt == 0, tt == TT - 1
                    nc.tensor.matmul(yt_a[:], lhsT=v_bf[:, tt, :], rhs=ets[tt][:, 0:512], start=st, stop=sp)
                    nc.tensor.matmul(yt_b[:], lhsT=v_bf[:, tt, :], rhs=ets[tt][:, 512:1024], start=st, stop=sp)
                # softmax sums via ones-matmul (half a then b through one slot each)
                recip = f4k.tile([1, S], f32, tag="f4k")
                for half, (lo, hi) in enumerate([(0, 512), (512, 1024)]):
                    sm = stp.tile([128, 512], f32, tag="stp")
                    for tt in range(TT):
                        nc.tensor.matmul(sm[0:1, :], lhsT=ones[:], rhs=ets[tt][:, lo:hi],
                                         start=tt == 0, stop=tt == TT - 1)
                    nc.vector.reciprocal(out=recip[:, lo:hi], in_=sm[0:1, :])
                bc = f4k.tile([128, S], f32, tag="f4k")
                nc.gpsimd.partition_broadcast(bc[:], recip[:])
                ybf = ybfp.tile([128, S], bf, tag="ybf")
                nc.vector.tensor_mul(out=ybf[:, 0:512], in0=yt_a[:], in1=bc[:, 0:512])
                nc.vector.tensor_mul(out=ybf[:, 512:1024], in0=yt_b[:], in1=bc[:, 512:1024])
                y_tiles[h] = ybf

            # interleave Q-projections ahead of attention to keep PE dense
            q_pass(0)
            q_pass(1)
            attention(0)
            q_pass(2)
            attention(1)
            q_pass(3)
            # prefetch wo weights while attention 2-3 run (hT slots free by now)
            wo_tiles = []
            woT_ap = woT.ap().rearrange("(f p) d -> f p d", f=4, p=128)
            for f in range(4):
                t = big.tile([128, 4 * S], bf, tag="big")  # same 8KB slot size
                nc.sync.dma_start(out=t[:], in_=woT_ap[f])
                wo_tiles.append(t)
            attention(2)
            attention(3)

            # ---- output projection (row-parallel partial) ----
            for dto in range(DT):
                pool = ytp if dto % 2 == 0 else stp
                yo_a = pool.tile([128, 512], f32, tag="ytp" if dto % 2 == 0 else "stp")
                yo_b = pool.tile([128, 512], f32, tag="ytp" if dto % 2 == 0 else "stp")
                for f in range(4):
                    lt = wo_tiles[f][:, dto * 128:(dto + 1) * 128]
                    st, sp = f == 0, f == 3
                    nc.tensor.matmul(yo_a[:], lhsT=lt, rhs=y_tiles[f][:, 0:512], start=st, stop=sp)
                    nc.tensor.matmul(yo_b[:], lhsT=lt, rhs=y_tiles[f][:, 512:1024], start=st, stop=sp)
                yo_sb = f4k.tile([128, S], f32, tag="f4k")
                nc.vector.tensor_copy(out=yo_sb[:, 0:512], in_=yo_a[:])
                nc.vector.tensor_copy(out=yo_sb[:, 512:1024], in_=yo_b[:])
                nc.sync.dma_start(out=yprt.ap()[dto * 128:(dto + 1) * 128, :], in_=yo_sb[:])

    nc.finalize()
    return nc


def _get_nc():
    if "nc" not in _BUILT:
        _BUILT["nc"] = _build()
    return _BUILT["nc"]


def _numpy_reference(hidden_states, freqs_cos, freqs_sin, atten_mask,
                     k_cache, v_cache, wq, wk, wv, wo):
    """Exact fp32 fallback (only used if atten_mask is nonzero)."""
    b, s, d = hidden_states.shape
    nh, hd, _ = wq.shape
    nkv = wk.shape[0]
    g = nh // nkv
    scale = float(hd) ** 0.5
    h = hidden_states.astype(np.float32)
    q = np.einsum("bsd,hed->bshe", h, wq)
    k = np.einsum("bsd,hed->bshe", h, wk)
    v = np.einsum("bsd,hed->bshe", h, wv)

    def rope(x):
        half = x.shape[-1] // 2
        xr, xi = x[..., :half], x[..., half:]
        c = freqs_cos[None, :, None, :]
        sn = freqs_sin[None, :, None, :]
        return np.concatenate([xr * c - xi * sn, xr * sn + xi * c], axis=-1)

    q, k = rope(q), rope(k)
    kh = np.concatenate([k_cache, k.transpose(0, 2, 3, 1)], axis=-1)
    vh = np.concatenate([v_cache, v.transpose(0, 2, 1, 3)], axis=2)
    qg = q.reshape(b, s, nkv, g, hd).transpose(0, 2, 3, 1, 4)
    scores = np.einsum("bngse,bnet->bngst", qg, kh) / scale + atten_mask
    m = scores.max(axis=-1, keepdims=True)
    p = np.exp(scores - m)
    p = p / p.sum(axis=-1, keepdims=True)
    yg = np.einsum("bngst,bnte->bngse", p, vh)
    y = yg.transpose(0, 3, 1, 2, 4).reshape(b, s, nh * hd)
    y = np.einsum("bsf,df->bsd", y, wo)
    return (y.astype(np.float32), kh.astype(np.float32), vh.astype(np.float32))


def kernel(hidden_states, freqs_cos, freqs_sin, atten_mask, k_cache, v_cache,
           wq, wk, wv, wo):
    hidden_states = np.asarray(hidden_states, np.float32)
    freqs_cos = np.asarray(freqs_cos, np.float32)
    freqs_sin = np.asarray(freqs_sin, np.float32)
    atten_mask = np.asarray(atten_mask, np.float32)
    k_cache = np.asarray(k_cache, np.float32)
    v_cache = np.asarray(v_cache, np.float32)
    wq = np.asarray(wq, np.float32)
    wk = np.asarray(wk, np.float32)
    wv = np.asarray(wv, np.float32)
    wo = np.asarray(wo, np.float32)

    if np.any(atten_mask):
        return _numpy_reference(hidden_states, freqs_cos, freqs_sin, atten_mask,
                                k_cache, v_cache, wq, wk, wv, wo)

    from concourse.bass_utils import run_bass_kernel_spmd

    nc = _get_nc()

    hT = np.ascontiguousarray(hidden_states[0].T).astype(BF16)        # [D, S]
    cosF = np.concatenate([freqs_cos.T, freqs_cos.T], axis=0).astype(np.float32)
    sinF = np.concatenate([-freqs_sin.T, freqs_sin.T], axis=0).astype(np.float32)

    in_maps = []
    for c in range(8):
        wqT = np.ascontiguousarray(
            wq[HQ * c:HQ * (c + 1)].transpose(2, 0, 1).reshape(D, HQ * HD)
        ).astype(BF16)
        wkT = np.ascontiguousarray(wk[c].T).astype(BF16)
        wvT = np.ascontiguousarray(wv[c].T).astype(BF16)
        woT = np.ascontiguousarray(wo[:, 512 * c:512 * (c + 1)].T).astype(BF16)
        in_maps.append({
            "hT": hT, "wqT": wqT, "wkT": wkT, "wvT": wvT,
            "kcache": np.ascontiguousarray(k_cache[0, c]),
            "vcache": np.ascontiguousarray(v_cache[0, c]),
            "woT": woT, "cosF": cosF, "sinF": sinF,
        })

    res = run_bass_kernel_spmd(nc, in_maps, core_ids=list(range(8)))

    y_acc = np.zeros((D, S), np.float32)
    kh = np.empty((1, NKV, HD, T), np.float32)
    vh = np.empty((1, NKV, T, HD), np.float32)
    kh[0, :, :, :CACHE] = k_cache[0]
    vh[0, :, :CACHE, :] = v_cache[0]
    for c in range(8):
        r = res.results[c]
        y_acc += r["yprt"]
        kh[0, c, :, CACHE:] = r["khn"]
        vh[0, c, CACHE:, :] = r["vhn"]
    y = np.ascontiguousarray(y_acc.T)[None]                           # [1, S, D]
    return (y, kh, vh)
